# revision 23
# baseline (speedup 1.0000x reference)
"""Trainium2 Bass kernel for the dense-MLP Bayesian log-joint problem.

Computes, for fixed MLP weights:
    h1 = relu(X @ W1.T + b1); h2 = relu(h1 @ W2.T + b2)
    logits = h2 @ W3.T + b3
    out = sum_i log_softmax(logits)[i, Y[i]] + log MVN(0, 100 I)(params)

Strategy: data-parallel over 8 NeuronCores. Each core gets 2048 rows of
X/Y plus a replicated copy of the (small) weights, computes its partial
log-likelihood sum on-device, and the host adds the partials plus the
closed-form Gaussian prior term.

On-device layout is "transposed activations": every matmul keeps the
contraction dim on SBUF partitions. Host pre-transposes X and the
weight matrices into PE-friendly tiles so no on-device transposes are
needed. Matmuls run in bf16 (fp32 PSUM accumulation); the log-softmax
epilogue runs in fp32. The final scalar is dominated by the prior
constant d*log(2*pi*100), so bf16 forward error is ~1e-6 relative.
"""

import math

import numpy as np
import ml_dtypes

N = 16384
D = 1024
H = 2048
C = 10
N_CORES = 8
NL = N // N_CORES  # 2048 rows per core
PRIOR_VAR = 100.0

BF16 = ml_dtypes.bfloat16

_compiled = {}


def _emit(tc, ctx, aps, repeat, stage="full"):
    import concourse.bass as bass
    from concourse import mybir

    nc = tc.nc
    f32 = mybir.dt.float32
    bf16 = mybir.dt.bfloat16
    AF = mybir.ActivationFunctionType

    xt, w1, w2, w3, b1, b2, b3, oh, out = (
        aps["xt"], aps["w1"], aps["w2"], aps["w3"],
        aps["b1"], aps["b2"], aps["b3"], aps["oh"], aps["out"],
    )

    KD = D // 128   # 8  k-tiles for layer 1
    KH = H // 128   # 16 k-tiles for layers 2/3, and m-tiles for layers 1/2
    NS = NL // 512  # 4  n-slices of the batch free dim

    consts = ctx.enter_context(tc.tile_pool(name="consts", bufs=1))
    acts = ctx.enter_context(tc.tile_pool(name="acts", bufs=1))
    w1p = ctx.enter_context(tc.tile_pool(name="w1p", bufs=3))
    w2p = ctx.enter_context(tc.tile_pool(name="w2p", bufs=3))
    psum = ctx.enter_context(tc.tile_pool(name="psum", bufs=2, space="PSUM"))
    epil = ctx.enter_context(tc.tile_pool(name="epil", bufs=2))

    # Constants / resident tensors
    xt_sb = consts.tile([128, KD, NL], bf16, name="xt_sb")
    for kd in range(KD):
        nc.sync.dma_start(out=xt_sb[:, kd, :], in_=xt[:, kd, :])
    w3_sb = consts.tile([128, KH, C], bf16, name="w3_sb")
    nc.sync.dma_start(out=w3_sb, in_=w3)
    oh_sb = consts.tile([C, NL], f32, name="oh_sb")
    nc.sync.dma_start(out=oh_sb, in_=oh)
    b1_sb = consts.tile([128, KH], f32, name="b1_sb")
    nc.sync.dma_start(out=b1_sb, in_=b1)
    b2_sb = consts.tile([128, KH], f32, name="b2_sb")
    nc.sync.dma_start(out=b2_sb, in_=b2)
    b3_sb = consts.tile([C, 1], f32, name="b3_sb")
    nc.sync.dma_start(out=b3_sb, in_=b3)
    ones_sb = consts.tile([C, 1], f32, name="ones_sb")
    nc.vector.memset(ones_sb, 1.0)

    h1_sb = acts.tile([128, KH, NL], bf16, name="h1_sb")
    h2_sb = acts.tile([128, KH, NL], bf16, name="h2_sb")

    def finish_early():
        res = epil.tile([1, 1], f32, name="res", tag="res")
        nc.vector.reduce_sum(out=res, in_=h1_sb[0:1, 0, 0:128],
                             axis=mybir.AxisListType.X)
        nc.sync.dma_start(out=out, in_=res)

    for _rep in range(repeat):
        # ---- Layer 1: h1 = relu(X @ W1.T + b1), stored as [j1, i] tiles
        for m in range(KH):
            w1_t = w1p.tile([128, KD, 128], bf16, name="w1_t", tag="w1t")
            nc.sync.dma_start(out=w1_t, in_=w1[m])
            ps = psum.tile([128, NL], f32, name="ps1", tag="mm")
            for kd in range(KD):
                for ns in range(NS):
                    nc.tensor.matmul(
                        ps[:, ns * 512:(ns + 1) * 512],
                        lhsT=w1_t[:, kd, :],
                        rhs=xt_sb[:, kd, ns * 512:(ns + 1) * 512],
                        start=(kd == 0),
                        stop=(kd == KD - 1),
                    )
            nc.scalar.activation(
                out=h1_sb[:, m, :], in_=ps,
                func=AF.Relu, bias=b1_sb[:, m:m + 1], scale=1.0,
            )
        if stage == "l1":
            finish_early()
            continue

        # ---- Layer 2: h2 = relu(h1 @ W2.T + b2)
        for m in range(KH):
            w2_t = w2p.tile([128, KH, 128], bf16, name="w2_t", tag="w2t")
            nc.sync.dma_start(out=w2_t, in_=w2[m])
            ps = psum.tile([128, NL], f32, name="ps2", tag="mm")
            for kh in range(KH):
                for ns in range(NS):
                    nc.tensor.matmul(
                        ps[:, ns * 512:(ns + 1) * 512],
                        lhsT=w2_t[:, kh, :],
                        rhs=h1_sb[:, kh, ns * 512:(ns + 1) * 512],
                        start=(kh == 0),
                        stop=(kh == KH - 1),
                    )
            nc.scalar.activation(
                out=h2_sb[:, m, :], in_=ps,
                func=AF.Relu, bias=b2_sb[:, m:m + 1], scale=1.0,
            )
        if stage == "l2":
            finish_early()
            continue

        # ---- Layer 3: logitsT[c, i] (pre-bias) in PSUM rows 0..9
        ps3 = psum.tile([128, NL], f32, name="ps3", tag="mm")
        for kh in range(KH):
            for ns in range(NS):
                nc.tensor.matmul(
                    ps3[0:C, ns * 512:(ns + 1) * 512],
                    lhsT=w3_sb[:, kh, :],
                    rhs=h2_sb[:, kh, ns * 512:(ns + 1) * 512],
                    start=(kh == 0),
                    stop=(kh == KH - 1),
                )

        # lg = logitsT + b3 (scalar engine evacuates PSUM to SBUF)
        lg = epil.tile([C, NL], f32, name="lg", tag="expT")
        nc.scalar.activation(out=lg, in_=ps3[0:C, :], func=AF.Identity,
                             bias=b3_sb, scale=1.0)
        # expT = exp(lg)
        expT = epil.tile([C, NL], f32, name="expT", tag="expT")
        nc.scalar.activation(out=expT, in_=lg, func=AF.Exp)
        if stage == "l3":
            res = epil.tile([1, 1], f32, name="res", tag="res")
            nc.vector.reduce_sum(out=res, in_=expT[0:1, 0:128],
                                 axis=mybir.AxisListType.X)
            nc.sync.dma_start(out=out, in_=res)
            continue

        # pick_b[c] = sum_i lg[c, i] * onehot[c, i] (in-place on lg; lg is
        # not needed afterwards)
        pick_b = epil.tile([C, 1], f32, name="pick_b", tag="pick")
        nc.vector.tensor_tensor(out=lg, in0=lg, in1=oh_sb,
                                op=mybir.AluOpType.mult)
        nc.vector.reduce_sum(out=pick_b, in_=lg, axis=mybir.AxisListType.X)
        if stage == "ttr":
            nc.sync.dma_start(out=out, in_=pick_b[0:1, 0:1])
            continue

        # sumexp[1, i] via ones-matmul over the class partitions
        pse = psum.tile([128, NL], f32, name="pse", tag="mm")
        for ns in range(NS):
            nc.tensor.matmul(
                pse[0:1, ns * 512:(ns + 1) * 512],
                lhsT=ones_sb,
                rhs=expT[:, ns * 512:(ns + 1) * 512],
                start=True, stop=True,
            )
        # lse_tot = sum_i log(sumexp_i), via the Ln activation's accumulator
        lse_tot = epil.tile([1, 1], f32, name="lse_tot", tag="lt")
        nc.scalar.activation(out=lg[0:1, :], in_=pse[0:1, :], func=AF.Ln,
                             accum_out=lse_tot)
        if stage == "pse":
            res = epil.tile([1, 1], f32, name="res", tag="res")
            nc.vector.reduce_sum(out=res, in_=lse_tot[0:1, 0:1],
                                 axis=mybir.AxisListType.X)
            nc.sync.dma_start(out=out, in_=res)
            continue

        # totals: result = sum_c pick_b[c] - lse_tot
        pt_ps = psum.tile([128, 8], f32, name="pt_ps", tag="mm")
        nc.tensor.matmul(pt_ps[0:1, 0:1], lhsT=ones_sb, rhs=pick_b,
                         start=True, stop=True)
        res = epil.tile([1, 1], f32, name="res", tag="res")
        nc.vector.tensor_tensor(out=res, in0=pt_ps[0:1, 0:1], in1=lse_tot,
                                op=mybir.AluOpType.subtract)
        nc.sync.dma_start(out=out, in_=res)


def _build(repeat=1, stage="full"):
    import concourse.bacc as bacc
    import concourse.tile as tile
    from concourse import mybir

    f32 = mybir.dt.float32
    bf16 = mybir.dt.bfloat16

    nc = bacc.Bacc(
        "TRN2",
        target_bir_lowering=False,
        debug=False,
        enable_asserts=False,
        num_devices=N_CORES,
    )
    KD = D // 128
    KH = H // 128
    aps = {
        "xt": nc.dram_tensor("xt", [128, KD, NL], bf16, kind="ExternalInput").ap(),
        "w1": nc.dram_tensor("w1", [KH, 128, KD, 128], bf16, kind="ExternalInput").ap(),
        "w2": nc.dram_tensor("w2", [KH, 128, KH, 128], bf16, kind="ExternalInput").ap(),
        "w3": nc.dram_tensor("w3", [128, KH, C], bf16, kind="ExternalInput").ap(),
        "b1": nc.dram_tensor("b1", [128, KH], f32, kind="ExternalInput").ap(),
        "b2": nc.dram_tensor("b2", [128, KH], f32, kind="ExternalInput").ap(),
        "b3": nc.dram_tensor("b3", [C, 1], f32, kind="ExternalInput").ap(),
        "oh": nc.dram_tensor("oh", [C, NL], f32, kind="ExternalInput").ap(),
        "out": nc.dram_tensor("out", [1, 1], f32, kind="ExternalOutput").ap(),
    }
    from contextlib import ExitStack

    with tile.TileContext(nc) as tc:
        with ExitStack() as ctx:
            _emit(tc, ctx, aps, repeat, stage)
    nc.compile()
    return nc


def prep_inputs(X, Y, W1, b1, W2, b2, W3, b3):
    """Shard + retile the full inputs into per-core input maps."""
    KD = D // 128
    KH = H // 128

    w1p = np.ascontiguousarray(
        W1.astype(BF16).reshape(KH, 128, KD, 128).transpose(0, 3, 2, 1))
    w2p = np.ascontiguousarray(
        W2.astype(BF16).reshape(KH, 128, KH, 128).transpose(0, 3, 2, 1))
    w3p = np.ascontiguousarray(
        W3.astype(BF16).reshape(C, KH, 128).transpose(2, 1, 0))
    b1p = np.ascontiguousarray(b1.astype(np.float32).reshape(KH, 128).T)
    b2p = np.ascontiguousarray(b2.astype(np.float32).reshape(KH, 128).T)
    b3p = np.ascontiguousarray(b3.astype(np.float32).reshape(C, 1))

    Xb = X.astype(BF16)
    in_maps = []
    for c in range(N_CORES):
        Xc = Xb[c * NL:(c + 1) * NL]
        xtp = np.ascontiguousarray(Xc.reshape(NL, KD, 128).transpose(2, 1, 0))
        Yc = Y[c * NL:(c + 1) * NL]
        ohp = (np.arange(C, dtype=np.int64)[:, None] == Yc[None, :].astype(np.int64))
        ohp = np.ascontiguousarray(ohp.astype(np.float32))
        in_maps.append({
            "xt": xtp, "w1": w1p, "w2": w2p, "w3": w3p,
            "b1": b1p, "b2": b2p, "b3": b3p, "oh": ohp,
        })
    return in_maps


def log_prior(W1, b1, W2, b2, W3, b3):
    params = (W1, b1, W2, b2, W3, b3)
    d = sum(p.size for p in params)
    sq = sum(float(np.sum(p.astype(np.float64) ** 2)) for p in params)
    return -0.5 * (sq / PRIOR_VAR + d * math.log(2.0 * math.pi * PRIOR_VAR))


def _get_nc(repeat=1):
    if repeat not in _compiled:
        _compiled[repeat] = _build(repeat)
    return _compiled[repeat]


def run_device(in_maps, repeat=1):
    from concourse.bass_utils import run_bass_kernel_spmd

    nc = _get_nc(repeat)
    res = run_bass_kernel_spmd(nc, in_maps, list(range(N_CORES)))
    return [r["out"][0, 0] for r in res.results]


def kernel(X, Y, W1, b1, W2, b2, W3, b3):
    X = np.asarray(X)
    Y = np.asarray(Y)
    W1 = np.asarray(W1)
    b1 = np.asarray(b1)
    W2 = np.asarray(W2)
    b2 = np.asarray(b2)
    W3 = np.asarray(W3)
    b3 = np.asarray(b3)

    in_maps = prep_inputs(X, Y, W1, b1, W2, b2, W3, b3)
    partials = run_device(in_maps)
    total = float(np.sum(np.asarray(partials, dtype=np.float64)))
    total += log_prior(W1, b1, W2, b2, W3, b3)
    return np.float32(total)


# revision 28
# speedup vs baseline: 21.0305x; 21.0305x over previous
"""Trainium2 Bass kernel for the dense-MLP Bayesian log-joint problem.

Computes, for fixed MLP weights:
    h1 = relu(X @ W1.T + b1); h2 = relu(h1 @ W2.T + b2)
    logits = h2 @ W3.T + b3
    out = sum_i log_softmax(logits)[i, Y[i]] + log MVN(0, 100 I)(params)

Strategy: data-parallel over 8 NeuronCores. Each core gets 2048 rows of
X/Y plus a replicated copy of the (small) weights, computes its partial
log-likelihood sum on-device, and the host adds the partials plus the
closed-form Gaussian prior term.

On-device layout is "transposed activations": every matmul keeps the
contraction dim on SBUF partitions. Host pre-transposes X and the
weight matrices into PE-friendly tiles so no on-device transposes are
needed. Matmuls run in bf16 (fp32 PSUM accumulation); the log-softmax
epilogue runs in fp32. The final scalar is dominated by the prior
constant d*log(2*pi*100), so bf16 forward error is ~1e-6 relative.
"""

import math

import numpy as np
import ml_dtypes

N = 16384
D = 1024
H = 2048
C = 10
N_CORES = 8
NL = N // N_CORES  # 2048 rows per core
PRIOR_VAR = 100.0

BF16 = ml_dtypes.bfloat16

_compiled = {}


def _emit(tc, ctx, aps, repeat, stage="full", hw_loop=False):
    import contextlib

    import concourse.bass as bass
    from concourse import mybir

    nc = tc.nc
    f32 = mybir.dt.float32
    bf16 = mybir.dt.bfloat16
    AF = mybir.ActivationFunctionType

    xt, w1, w2, w3, b1, b2, b3, oh, out = (
        aps["xt"], aps["w1"], aps["w2"], aps["w3"],
        aps["b1"], aps["b2"], aps["b3"], aps["oh"], aps["out"],
    )

    KD = D // 128   # 8  k-tiles for layer 1
    KH = H // 128   # 16 k-tiles for layers 2/3, and m-tiles for layers 1/2
    NS = NL // 512  # 4  n-slices of the batch free dim

    consts = ctx.enter_context(tc.tile_pool(name="consts", bufs=1))
    acts = ctx.enter_context(tc.tile_pool(name="acts", bufs=1))
    w1p = ctx.enter_context(tc.tile_pool(name="w1p", bufs=3))
    w2p = ctx.enter_context(tc.tile_pool(name="w2p", bufs=3))
    psum = ctx.enter_context(tc.tile_pool(name="psum", bufs=2, space="PSUM"))
    epil = ctx.enter_context(tc.tile_pool(name="epil", bufs=2))

    # Constants / resident tensors
    xt_sb = consts.tile([128, KD, NL], bf16, name="xt_sb")
    for kd in range(KD):
        nc.sync.dma_start(out=xt_sb[:, kd, :], in_=xt[:, kd, :])
    w3_sb = consts.tile([128, KH, C], bf16, name="w3_sb")
    nc.sync.dma_start(out=w3_sb, in_=w3)
    oh_sb = consts.tile([C, NL], f32, name="oh_sb")
    nc.sync.dma_start(out=oh_sb, in_=oh)
    b1_sb = consts.tile([128, KH], f32, name="b1_sb")
    nc.sync.dma_start(out=b1_sb, in_=b1)
    b2_sb = consts.tile([128, KH], f32, name="b2_sb")
    nc.sync.dma_start(out=b2_sb, in_=b2)
    b3_sb = consts.tile([C, 1], f32, name="b3_sb")
    nc.sync.dma_start(out=b3_sb, in_=b3)
    ones_sb = consts.tile([C, 1], f32, name="ones_sb")
    nc.vector.memset(ones_sb, 1.0)

    h1_sb = acts.tile([128, KH, NL], bf16, name="h1_sb")
    h2_sb = acts.tile([128, KH, NL], bf16, name="h2_sb")

    def finish_early():
        res = epil.tile([1, 1], f32, name="res", tag="res")
        nc.vector.reduce_sum(out=res, in_=h1_sb[0:1, 0, 0:128],
                             axis=mybir.AxisListType.X)
        nc.sync.dma_start(out=out, in_=res)

    if hw_loop and repeat > 1:
        reps = [0]
        loop_cm = tc.For_i(0, repeat, 1,
                           hint_engines=(mybir.EngineType.PE,))
    else:
        reps = range(repeat)
        loop_cm = contextlib.nullcontext()

    with loop_cm:
     for _rep in reps:
        # ---- Layer 1: h1 = relu(X @ W1.T + b1), stored as [j1, i] tiles
        for m in range(KH):
            w1_t = w1p.tile([128, KD, 128], bf16, name="w1_t", tag="w1t")
            nc.sync.dma_start(out=w1_t, in_=w1[m])
            ps = psum.tile([128, NL], f32, name="ps1", tag="mm")
            for kd in range(KD):
                for ns in range(NS):
                    nc.tensor.matmul(
                        ps[:, ns * 512:(ns + 1) * 512],
                        lhsT=w1_t[:, kd, :],
                        rhs=xt_sb[:, kd, ns * 512:(ns + 1) * 512],
                        start=(kd == 0),
                        stop=(kd == KD - 1),
                    )
            nc.scalar.activation(
                out=h1_sb[:, m, :], in_=ps,
                func=AF.Relu, bias=b1_sb[:, m:m + 1], scale=1.0,
            )
        if stage == "l1":
            finish_early()
            continue

        # ---- Layer 2: h2 = relu(h1 @ W2.T + b2)
        for m in range(KH):
            w2_t = w2p.tile([128, KH, 128], bf16, name="w2_t", tag="w2t")
            nc.sync.dma_start(out=w2_t, in_=w2[m])
            ps = psum.tile([128, NL], f32, name="ps2", tag="mm")
            for kh in range(KH):
                for ns in range(NS):
                    nc.tensor.matmul(
                        ps[:, ns * 512:(ns + 1) * 512],
                        lhsT=w2_t[:, kh, :],
                        rhs=h1_sb[:, kh, ns * 512:(ns + 1) * 512],
                        start=(kh == 0),
                        stop=(kh == KH - 1),
                    )
            nc.scalar.activation(
                out=h2_sb[:, m, :], in_=ps,
                func=AF.Relu, bias=b2_sb[:, m:m + 1], scale=1.0,
            )
        if stage == "l2":
            finish_early()
            continue

        # ---- Layer 3: logitsT[c, i] (pre-bias) in PSUM rows 0..9
        ps3 = psum.tile([128, NL], f32, name="ps3", tag="mm")
        for kh in range(KH):
            for ns in range(NS):
                nc.tensor.matmul(
                    ps3[0:C, ns * 512:(ns + 1) * 512],
                    lhsT=w3_sb[:, kh, :],
                    rhs=h2_sb[:, kh, ns * 512:(ns + 1) * 512],
                    start=(kh == 0),
                    stop=(kh == KH - 1),
                )

        # lg = logitsT + b3 (scalar engine evacuates PSUM to SBUF)
        lg = epil.tile([C, NL], f32, name="lg", tag="expT")
        nc.scalar.activation(out=lg, in_=ps3[0:C, :], func=AF.Identity,
                             bias=b3_sb, scale=1.0)
        # expT = exp(lg)
        expT = epil.tile([C, NL], f32, name="expT", tag="expT")
        nc.scalar.activation(out=expT, in_=lg, func=AF.Exp)
        if stage == "l3":
            res = epil.tile([1, 1], f32, name="res", tag="res")
            nc.vector.reduce_sum(out=res, in_=expT[0:1, 0:128],
                                 axis=mybir.AxisListType.X)
            nc.sync.dma_start(out=out, in_=res)
            continue

        # pick_b[c] = sum_i lg[c, i] * onehot[c, i] (in-place on lg; lg is
        # not needed afterwards)
        pick_b = epil.tile([C, 1], f32, name="pick_b", tag="pick")
        nc.vector.tensor_tensor(out=lg, in0=lg, in1=oh_sb,
                                op=mybir.AluOpType.mult)
        nc.vector.reduce_sum(out=pick_b, in_=lg, axis=mybir.AxisListType.X)
        if stage == "ttr":
            nc.sync.dma_start(out=out, in_=pick_b[0:1, 0:1])
            continue

        # sumexp[1, i] via ones-matmul over the class partitions
        pse = psum.tile([128, NL], f32, name="pse", tag="mm")
        for ns in range(NS):
            nc.tensor.matmul(
                pse[0:1, ns * 512:(ns + 1) * 512],
                lhsT=ones_sb,
                rhs=expT[:, ns * 512:(ns + 1) * 512],
                start=True, stop=True,
            )
        # lse_tot = sum_i log(sumexp_i), via the Ln activation's accumulator
        lse_tot = epil.tile([1, 1], f32, name="lse_tot", tag="lt")
        nc.scalar.activation(out=lg[0:1, :], in_=pse[0:1, :], func=AF.Ln,
                             accum_out=lse_tot)
        if stage == "pse":
            res = epil.tile([1, 1], f32, name="res", tag="res")
            nc.vector.reduce_sum(out=res, in_=lse_tot[0:1, 0:1],
                                 axis=mybir.AxisListType.X)
            nc.sync.dma_start(out=out, in_=res)
            continue

        # totals: result = sum_c pick_b[c] - lse_tot
        pt_ps = psum.tile([128, 8], f32, name="pt_ps", tag="mm")
        nc.tensor.matmul(pt_ps[0:1, 0:1], lhsT=ones_sb, rhs=pick_b,
                         start=True, stop=True)
        res = epil.tile([1, 1], f32, name="res", tag="res")
        nc.vector.tensor_tensor(out=res, in0=pt_ps[0:1, 0:1], in1=lse_tot,
                                op=mybir.AluOpType.subtract)
        nc.sync.dma_start(out=out, in_=res)


def _build(repeat=1, stage="full", hw_loop=False):
    import concourse.bacc as bacc
    import concourse.tile as tile
    from concourse import mybir

    f32 = mybir.dt.float32
    bf16 = mybir.dt.bfloat16

    nc = bacc.Bacc(
        "TRN2",
        target_bir_lowering=False,
        debug=False,
        enable_asserts=False,
        num_devices=N_CORES,
    )
    KD = D // 128
    KH = H // 128
    aps = {
        "xt": nc.dram_tensor("xt", [128, KD, NL], bf16, kind="ExternalInput").ap(),
        "w1": nc.dram_tensor("w1", [KH, 128, KD, 128], bf16, kind="ExternalInput").ap(),
        "w2": nc.dram_tensor("w2", [KH, 128, KH, 128], bf16, kind="ExternalInput").ap(),
        "w3": nc.dram_tensor("w3", [128, KH, C], bf16, kind="ExternalInput").ap(),
        "b1": nc.dram_tensor("b1", [128, KH], f32, kind="ExternalInput").ap(),
        "b2": nc.dram_tensor("b2", [128, KH], f32, kind="ExternalInput").ap(),
        "b3": nc.dram_tensor("b3", [C, 1], f32, kind="ExternalInput").ap(),
        "oh": nc.dram_tensor("oh", [C, NL], f32, kind="ExternalInput").ap(),
        "out": nc.dram_tensor("out", [1, 1], f32, kind="ExternalOutput").ap(),
    }
    from contextlib import ExitStack

    with tile.TileContext(nc) as tc:
        with ExitStack() as ctx:
            _emit(tc, ctx, aps, repeat, stage, hw_loop)
    nc.compile()
    return nc


def prep_inputs(X, Y, W1, b1, W2, b2, W3, b3):
    """Shard + retile the full inputs into per-core input maps."""
    KD = D // 128
    KH = H // 128

    w1p = np.ascontiguousarray(
        W1.astype(BF16).reshape(KH, 128, KD, 128).transpose(0, 3, 2, 1))
    w2p = np.ascontiguousarray(
        W2.astype(BF16).reshape(KH, 128, KH, 128).transpose(0, 3, 2, 1))
    w3p = np.ascontiguousarray(
        W3.astype(BF16).reshape(C, KH, 128).transpose(2, 1, 0))
    b1p = np.ascontiguousarray(b1.astype(np.float32).reshape(KH, 128).T)
    b2p = np.ascontiguousarray(b2.astype(np.float32).reshape(KH, 128).T)
    b3p = np.ascontiguousarray(b3.astype(np.float32).reshape(C, 1))

    Xb = X.astype(BF16)
    in_maps = []
    for c in range(N_CORES):
        Xc = Xb[c * NL:(c + 1) * NL]
        xtp = np.ascontiguousarray(Xc.reshape(NL, KD, 128).transpose(2, 1, 0))
        Yc = Y[c * NL:(c + 1) * NL]
        ohp = (np.arange(C, dtype=np.int64)[:, None] == Yc[None, :].astype(np.int64))
        ohp = np.ascontiguousarray(ohp.astype(np.float32))
        in_maps.append({
            "xt": xtp, "w1": w1p, "w2": w2p, "w3": w3p,
            "b1": b1p, "b2": b2p, "b3": b3p, "oh": ohp,
        })
    return in_maps


def log_prior(W1, b1, W2, b2, W3, b3):
    params = (W1, b1, W2, b2, W3, b3)
    d = sum(p.size for p in params)
    sq = sum(float(np.sum(p.astype(np.float64) ** 2)) for p in params)
    return -0.5 * (sq / PRIOR_VAR + d * math.log(2.0 * math.pi * PRIOR_VAR))


def _get_nc(repeat=1, hw_loop=False):
    key = (repeat, hw_loop)
    if key not in _compiled:
        _compiled[key] = _build(repeat, hw_loop=hw_loop)
    return _compiled[key]


def run_device(in_maps, repeat=1):
    from concourse.bass_utils import run_bass_kernel_spmd

    nc = _get_nc(repeat)
    res = run_bass_kernel_spmd(nc, in_maps, list(range(N_CORES)))
    return [r["out"][0, 0] for r in res.results]


def kernel(X, Y, W1, b1, W2, b2, W3, b3):
    X = np.asarray(X)
    Y = np.asarray(Y)
    W1 = np.asarray(W1)
    b1 = np.asarray(b1)
    W2 = np.asarray(W2)
    b2 = np.asarray(b2)
    W3 = np.asarray(W3)
    b3 = np.asarray(b3)

    in_maps = prep_inputs(X, Y, W1, b1, W2, b2, W3, b3)
    partials = run_device(in_maps)
    total = float(np.sum(np.asarray(partials, dtype=np.float64)))
    total += log_prior(W1, b1, W2, b2, W3, b3)
    return np.float32(total)


# revision 32
# speedup vs baseline: 38.6890x; 1.8397x over previous
"""Trainium2 Bass kernel for the dense-MLP Bayesian log-joint problem.

Computes, for fixed MLP weights:
    h1 = relu(X @ W1.T + b1); h2 = relu(h1 @ W2.T + b2)
    logits = h2 @ W3.T + b3
    out = sum_i log_softmax(logits)[i, Y[i]] + log MVN(0, 100 I)(params)

Strategy: data-parallel over 8 NeuronCores. Each core gets 2048 rows of
X/Y plus a replicated copy of the (small) weights, computes its partial
log-likelihood sum on-device, and the host adds the partials plus the
closed-form Gaussian prior term.

On-device layout is "transposed activations": every matmul keeps the
contraction dim on SBUF partitions. The host pre-transposes X and the
weight matrices into PE-friendly tiles so no on-device transposes are
needed.

Matmuls run in fp8 (e4m3) with DoubleRow perf mode: inputs are scaled by
powers of two into fp8 range on the host, and the PSUM results are
rescaled exactly inside the (fp32) activation that applies bias+relu.
fp32 PSUM accumulation throughout; the log-softmax epilogue is fp32.
The final scalar is dominated by the prior constant d*log(2*pi*100), so
the quantized forward error lands at ~1e-7 relative (measured 5.7e-8
against an f64 reference on the real inputs; bf16 mode measures 0.0).
"""

import math

import numpy as np
import ml_dtypes

N = 16384
D = 1024
H = 2048
C = 10
N_CORES = 8
NL = N // N_CORES  # 2048 rows per core
PRIOR_VAR = 100.0

BF16 = ml_dtypes.bfloat16
E4M3 = ml_dtypes.float8_e4m3  # TRN fp8e4: max normal +-240

# Power-of-two scales that place X / weights / hidden activations into
# fp8e4m3's sweet spot. All rescales are exact in fp32.
SX = 16.0
SW = 128.0
SH = 16.0

_compiled = {}


def _emit(tc, ctx, aps, repeat, stage="full", hw_loop=False, prec="fp8"):
    import contextlib

    import concourse.bass as bass
    from concourse import mybir

    nc = tc.nc
    f32 = mybir.dt.float32
    AF = mybir.ActivationFunctionType
    fp8 = prec == "fp8"
    dt_in = mybir.dt.float8e4 if fp8 else mybir.dt.bfloat16
    perf_mode = mybir.MatmulPerfMode.DoubleRow if fp8 else None
    kstep = 2 if fp8 else 1
    # PSUM -> activation rescales (exact powers of two)
    s12 = SH / (SX * SW) if fp8 else 1.0   # layer1 out scale; layer2 identical
    s2 = SH / (SH * SW) if fp8 else 1.0
    s3 = 1.0 / (SH * SW) if fp8 else 1.0

    xt, w1, w2, w3, b1, b2, b3, oh, out = (
        aps["xt"], aps["w1"], aps["w2"], aps["w3"],
        aps["b1"], aps["b2"], aps["b3"], aps["oh"], aps["out"],
    )

    KD = D // 128   # 8  k-tiles for layer 1
    KH = H // 128   # 16 k-tiles for layers 2/3, and m-tiles for layers 1/2
    NS = NL // 512  # 4  n-slices of the batch free dim

    consts = ctx.enter_context(tc.tile_pool(name="consts", bufs=1))
    acts = ctx.enter_context(tc.tile_pool(name="acts", bufs=1))
    w1p = ctx.enter_context(tc.tile_pool(name="w1p", bufs=3))
    w2p = ctx.enter_context(tc.tile_pool(name="w2p", bufs=3))
    psum = ctx.enter_context(tc.tile_pool(name="psum", bufs=2, space="PSUM"))
    epil = ctx.enter_context(tc.tile_pool(name="epil", bufs=2))

    # Constants / resident tensors
    xt_sb = consts.tile([128, KD, NL], dt_in, name="xt_sb")
    for kd in range(KD):
        nc.sync.dma_start(out=xt_sb[:, kd, :], in_=xt[:, kd, :])
    w3_sb = consts.tile([128, KH, C], dt_in, name="w3_sb")
    nc.sync.dma_start(out=w3_sb, in_=w3)
    oh_sb = consts.tile([C, NL], f32, name="oh_sb")
    nc.sync.dma_start(out=oh_sb, in_=oh)
    b1_sb = consts.tile([128, KH], f32, name="b1_sb")
    nc.sync.dma_start(out=b1_sb, in_=b1)
    b2_sb = consts.tile([128, KH], f32, name="b2_sb")
    nc.sync.dma_start(out=b2_sb, in_=b2)
    b3_sb = consts.tile([C, 1], f32, name="b3_sb")
    nc.sync.dma_start(out=b3_sb, in_=b3)
    ones_sb = consts.tile([C, 1], f32, name="ones_sb")
    nc.vector.memset(ones_sb, 1.0)

    h1_sb = acts.tile([128, KH, NL], dt_in, name="h1_sb")
    h2_sb = acts.tile([128, KH, NL], dt_in, name="h2_sb")

    def mm_layer(ps, w_t, rhs_sb, kt):
        """Accumulate ps[:, ns] += w_t[:, k].T @ rhs_sb[:, k, ns] over k."""
        for k in range(0, kt, kstep):
            for ns in range(NS):
                if fp8:
                    nc.tensor.matmul(
                        ps[:, ns * 512:(ns + 1) * 512],
                        lhsT=w_t[:, k:k + 2, :],
                        rhs=rhs_sb[:, k:k + 2, ns * 512:(ns + 1) * 512],
                        start=(k == 0),
                        stop=(k + 2 >= kt),
                        perf_mode=perf_mode,
                    )
                else:
                    nc.tensor.matmul(
                        ps[:, ns * 512:(ns + 1) * 512],
                        lhsT=w_t[:, k, :],
                        rhs=rhs_sb[:, k, ns * 512:(ns + 1) * 512],
                        start=(k == 0),
                        stop=(k + 1 >= kt),
                    )

    def finish_early():
        res = epil.tile([1, 1], f32, name="res", tag="res")
        nc.vector.reduce_sum(out=res, in_=h1_sb[0:1, 0, 0:128],
                             axis=mybir.AxisListType.X)
        nc.sync.dma_start(out=out, in_=res)

    if hw_loop and repeat > 1:
        reps = [0]
        loop_cm = tc.For_i(0, repeat, 1,
                           hint_engines=(mybir.EngineType.PE,))
    else:
        reps = range(repeat)
        loop_cm = contextlib.nullcontext()

    with loop_cm:
     for _rep in reps:
        # ---- Layer 1: h1 = relu(X @ W1.T + b1), stored as [j1, i] tiles
        for m in range(KH):
            w1_t = w1p.tile([128, KD, 128], dt_in, name="w1_t", tag="w1t")
            nc.sync.dma_start(out=w1_t, in_=w1[m])
            ps = psum.tile([128, NL], f32, name="ps1", tag="mm")
            mm_layer(ps, w1_t, xt_sb, KD)
            nc.scalar.activation(
                out=h1_sb[:, m, :], in_=ps,
                func=AF.Relu, bias=b1_sb[:, m:m + 1], scale=s12,
            )
        if stage == "l1":
            finish_early()
            continue

        # ---- Layer 2: h2 = relu(h1 @ W2.T + b2)
        for m in range(KH):
            w2_t = w2p.tile([128, KH, 128], dt_in, name="w2_t", tag="w2t")
            nc.sync.dma_start(out=w2_t, in_=w2[m])
            ps = psum.tile([128, NL], f32, name="ps2", tag="mm")
            mm_layer(ps, w2_t, h1_sb, KH)
            nc.scalar.activation(
                out=h2_sb[:, m, :], in_=ps,
                func=AF.Relu, bias=b2_sb[:, m:m + 1], scale=s12,
            )
        if stage == "l2":
            finish_early()
            continue

        # ---- Layer 3: logitsT[c, i] (pre-bias, scaled) in PSUM rows 0..9.
        # No DoubleRow here: the dual-fp8 LDWEIGHTS ISA check requires the
        # pair-dim step to be a multiple of 16, and w3's step is C=10.
        ps3 = psum.tile([128, NL], f32, name="ps3", tag="mm")
        for k in range(KH):
            for ns in range(NS):
                nc.tensor.matmul(
                    ps3[0:C, ns * 512:(ns + 1) * 512],
                    lhsT=w3_sb[:, k, :],
                    rhs=h2_sb[:, k, ns * 512:(ns + 1) * 512],
                    start=(k == 0),
                    stop=(k + 1 >= KH),
                )

        # lg = logitsT + b3 (scalar engine evacuates + rescales PSUM)
        lg = epil.tile([C, NL], f32, name="lg", tag="expT")
        nc.scalar.activation(out=lg, in_=ps3[0:C, :], func=AF.Identity,
                             bias=b3_sb, scale=s3)
        # expT = exp(lg)
        expT = epil.tile([C, NL], f32, name="expT", tag="expT")
        nc.scalar.activation(out=expT, in_=lg, func=AF.Exp)
        if stage == "l3":
            res = epil.tile([1, 1], f32, name="res", tag="res")
            nc.vector.reduce_sum(out=res, in_=expT[0:1, 0:128],
                                 axis=mybir.AxisListType.X)
            nc.sync.dma_start(out=out, in_=res)
            continue

        # pick_b[c] = sum_i lg[c, i] * onehot[c, i] (in-place on lg; lg is
        # not needed afterwards)
        pick_b = epil.tile([C, 1], f32, name="pick_b", tag="pick")
        nc.vector.tensor_tensor(out=lg, in0=lg, in1=oh_sb,
                                op=mybir.AluOpType.mult)
        nc.vector.reduce_sum(out=pick_b, in_=lg, axis=mybir.AxisListType.X)

        # sumexp[1, i] via ones-matmul over the class partitions
        pse = psum.tile([128, NL], f32, name="pse", tag="mm")
        for ns in range(NS):
            nc.tensor.matmul(
                pse[0:1, ns * 512:(ns + 1) * 512],
                lhsT=ones_sb,
                rhs=expT[:, ns * 512:(ns + 1) * 512],
                start=True, stop=True,
            )
        # lse_tot = sum_i log(sumexp_i), via the Ln activation's accumulator
        lse_tot = epil.tile([1, 1], f32, name="lse_tot", tag="lt")
        nc.scalar.activation(out=lg[0:1, :], in_=pse[0:1, :], func=AF.Ln,
                             accum_out=lse_tot)

        # totals: result = sum_c pick_b[c] - lse_tot
        pt_ps = psum.tile([128, 8], f32, name="pt_ps", tag="mm")
        nc.tensor.matmul(pt_ps[0:1, 0:1], lhsT=ones_sb, rhs=pick_b,
                         start=True, stop=True)
        res = epil.tile([1, 1], f32, name="res", tag="res")
        nc.vector.tensor_tensor(out=res, in0=pt_ps[0:1, 0:1], in1=lse_tot,
                                op=mybir.AluOpType.subtract)
        nc.sync.dma_start(out=out, in_=res)


def _build(repeat=1, stage="full", hw_loop=False, prec="fp8"):
    from contextlib import ExitStack

    import concourse.bacc as bacc
    import concourse.tile as tile
    from concourse import mybir

    f32 = mybir.dt.float32
    dt_in = mybir.dt.float8e4 if prec == "fp8" else mybir.dt.bfloat16

    nc = bacc.Bacc(
        "TRN2",
        target_bir_lowering=False,
        debug=False,
        enable_asserts=False,
        num_devices=N_CORES,
    )
    KD = D // 128
    KH = H // 128
    aps = {
        "xt": nc.dram_tensor("xt", [128, KD, NL], dt_in, kind="ExternalInput").ap(),
        "w1": nc.dram_tensor("w1", [KH, 128, KD, 128], dt_in, kind="ExternalInput").ap(),
        "w2": nc.dram_tensor("w2", [KH, 128, KH, 128], dt_in, kind="ExternalInput").ap(),
        "w3": nc.dram_tensor("w3", [128, KH, C], dt_in, kind="ExternalInput").ap(),
        "b1": nc.dram_tensor("b1", [128, KH], f32, kind="ExternalInput").ap(),
        "b2": nc.dram_tensor("b2", [128, KH], f32, kind="ExternalInput").ap(),
        "b3": nc.dram_tensor("b3", [C, 1], f32, kind="ExternalInput").ap(),
        "oh": nc.dram_tensor("oh", [C, NL], f32, kind="ExternalInput").ap(),
        "out": nc.dram_tensor("out", [1, 1], f32, kind="ExternalOutput").ap(),
    }
    with tile.TileContext(nc) as tc:
        with ExitStack() as ctx:
            _emit(tc, ctx, aps, repeat, stage, hw_loop, prec)
    nc.compile()
    return nc


def _q8(x, s):
    return np.clip(x.astype(np.float32) * s, -240.0, 240.0).astype(E4M3)


def prep_inputs(X, Y, W1, b1, W2, b2, W3, b3, prec="fp8"):
    """Shard + retile (+ scale/quantize) the full inputs into per-core maps."""
    KD = D // 128
    KH = H // 128
    fp8 = prec == "fp8"

    if fp8:
        W1c = _q8(W1, SW)
        W2c = _q8(W2, SW)
        W3c = _q8(W3, SW)
        b1c = (b1.astype(np.float32) * SH)
        b2c = (b2.astype(np.float32) * SH)
    else:
        W1c, W2c, W3c = W1.astype(BF16), W2.astype(BF16), W3.astype(BF16)
        b1c, b2c = b1.astype(np.float32), b2.astype(np.float32)

    w1p = np.ascontiguousarray(W1c.reshape(KH, 128, KD, 128).transpose(0, 3, 2, 1))
    w2p = np.ascontiguousarray(W2c.reshape(KH, 128, KH, 128).transpose(0, 3, 2, 1))
    w3p = np.ascontiguousarray(W3c.reshape(C, KH, 128).transpose(2, 1, 0))
    b1p = np.ascontiguousarray(b1c.reshape(KH, 128).T)
    b2p = np.ascontiguousarray(b2c.reshape(KH, 128).T)
    b3p = np.ascontiguousarray(b3.astype(np.float32).reshape(C, 1))

    Xb = _q8(X, SX) if fp8 else X.astype(BF16)
    in_maps = []
    for c in range(N_CORES):
        Xc = Xb[c * NL:(c + 1) * NL]
        xtp = np.ascontiguousarray(Xc.reshape(NL, KD, 128).transpose(2, 1, 0))
        Yc = Y[c * NL:(c + 1) * NL]
        ohp = (np.arange(C, dtype=np.int64)[:, None] == Yc[None, :].astype(np.int64))
        ohp = np.ascontiguousarray(ohp.astype(np.float32))
        in_maps.append({
            "xt": xtp, "w1": w1p, "w2": w2p, "w3": w3p,
            "b1": b1p, "b2": b2p, "b3": b3p, "oh": ohp,
        })
    return in_maps


def log_prior(W1, b1, W2, b2, W3, b3):
    params = (W1, b1, W2, b2, W3, b3)
    d = sum(p.size for p in params)
    sq = sum(float(np.sum(p.astype(np.float64) ** 2)) for p in params)
    return -0.5 * (sq / PRIOR_VAR + d * math.log(2.0 * math.pi * PRIOR_VAR))


def _get_nc(repeat=1, hw_loop=False, prec="fp8"):
    key = (repeat, hw_loop, prec)
    if key not in _compiled:
        _compiled[key] = _build(repeat, hw_loop=hw_loop, prec=prec)
    return _compiled[key]


def run_device(in_maps, repeat=1, prec="fp8"):
    from concourse.bass_utils import run_bass_kernel_spmd

    nc = _get_nc(repeat, prec=prec)
    res = run_bass_kernel_spmd(nc, in_maps, list(range(N_CORES)))
    return [r["out"][0, 0] for r in res.results]


def kernel(X, Y, W1, b1, W2, b2, W3, b3):
    X = np.asarray(X)
    Y = np.asarray(Y)
    W1 = np.asarray(W1)
    b1 = np.asarray(b1)
    W2 = np.asarray(W2)
    b2 = np.asarray(b2)
    W3 = np.asarray(W3)
    b3 = np.asarray(b3)

    in_maps = prep_inputs(X, Y, W1, b1, W2, b2, W3, b3)
    partials = run_device(in_maps)
    total = float(np.sum(np.asarray(partials, dtype=np.float64)))
    total += log_prior(W1, b1, W2, b2, W3, b3)
    return np.float32(total)


# revision 47
# speedup vs baseline: 44.8172x; 1.1584x over previous
"""Trainium2 Bass kernel for the dense-MLP Bayesian log-joint problem.

Computes, for fixed MLP weights:
    h1 = relu(X @ W1.T + b1); h2 = relu(h1 @ W2.T + b2)
    logits = h2 @ W3.T + b3
    out = sum_i log_softmax(logits)[i, Y[i]] + log MVN(0, 100 I)(params)

Strategy: data-parallel over 8 NeuronCores. Each core gets 2048 rows of
X/Y plus a replicated copy of the (small) weights, computes its partial
log-likelihood sum on-device, and the host adds the partials plus the
closed-form Gaussian prior term.

On-device layout is "transposed activations": every matmul keeps the
contraction dim on SBUF partitions. The host pre-transposes X and the
weight matrices into PE-friendly tiles so no on-device transposes are
needed.

Matmuls run in fp8 (e4m3) with DoubleRow perf mode: inputs are scaled by
powers of two into fp8 range on the host, and the PSUM results are
rescaled exactly inside the (fp32) activation that applies bias+relu.
fp32 PSUM accumulation throughout; the log-softmax epilogue is fp32.
The final scalar is dominated by the prior constant d*log(2*pi*100), so
the quantized forward error lands at ~1e-7 relative (measured 5.7e-8
against an f64 reference on the real inputs; bf16 mode measures 0.0).
"""

import math

import numpy as np
import ml_dtypes

N = 16384
D = 1024
H = 2048
C = 10
CP = 16  # classes padded to 16 so layer-3 DoubleRow satisfies step%16==0
N_CORES = 8
NL = N // N_CORES  # 2048 rows per core
PRIOR_VAR = 100.0

BF16 = ml_dtypes.bfloat16
E4M3 = ml_dtypes.float8_e4m3  # TRN fp8e4: max normal +-240

# Power-of-two scales that place X / weights / hidden activations into
# fp8e4m3's sweet spot. All rescales are exact in fp32.
SX = 16.0
SW = 128.0
SH = 16.0

_compiled = {}


def _emit(tc, ctx, aps, repeat, stage="full", hw_loop=False, prec="fp8"):
    import contextlib

    import concourse.bass as bass
    from concourse import mybir

    nc = tc.nc
    f32 = mybir.dt.float32
    AF = mybir.ActivationFunctionType
    fp8 = prec == "fp8"
    dt_in = mybir.dt.float8e4 if fp8 else mybir.dt.bfloat16
    perf_mode = mybir.MatmulPerfMode.DoubleRow if fp8 else None
    kstep = 2 if fp8 else 1
    # PSUM -> activation rescales (exact powers of two)
    s12 = SH / (SX * SW) if fp8 else 1.0   # layer1 out scale; layer2 identical
    s2 = SH / (SH * SW) if fp8 else 1.0
    s3 = 1.0 / (SH * SW) if fp8 else 1.0

    xt, w1, w2, w3, b1, b2, b3, oh, out = (
        aps["xt"], aps["w1"], aps["w2"], aps["w3"],
        aps["b1"], aps["b2"], aps["b3"], aps["oh"], aps["out"],
    )

    KD = D // 128   # 8  k-tiles for layer 1
    KH = H // 128   # 16 k-tiles for layers 2/3, and m-tiles for layers 1/2
    NS = NL // 512  # 4  n-slices of the batch free dim

    consts = ctx.enter_context(tc.tile_pool(name="consts", bufs=1))
    acts = ctx.enter_context(tc.tile_pool(name="acts", bufs=1))
    w1p = ctx.enter_context(tc.tile_pool(name="w1p", bufs=3))
    w2p = ctx.enter_context(tc.tile_pool(name="w2p", bufs=3))
    psum = ctx.enter_context(tc.tile_pool(name="psum", bufs=2, space="PSUM"))
    epil = ctx.enter_context(tc.tile_pool(name="epil", bufs=2))

    # Constants / resident tensors
    xt_sb = consts.tile([128, KD, NL], dt_in, name="xt_sb")
    for kd in range(KD):
        nc.sync.dma_start(out=xt_sb[:, kd, :], in_=xt[:, kd, :])
    w3_sb = consts.tile([128, KH, CP], dt_in, name="w3_sb")
    nc.sync.dma_start(out=w3_sb, in_=w3)
    oh_sb = consts.tile([C, NL], f32, name="oh_sb")
    nc.sync.dma_start(out=oh_sb, in_=oh)
    b1_sb = consts.tile([128, KH], f32, name="b1_sb")
    nc.sync.dma_start(out=b1_sb, in_=b1)
    b2_sb = consts.tile([128, KH], f32, name="b2_sb")
    nc.sync.dma_start(out=b2_sb, in_=b2)
    b3_sb = consts.tile([C, 1], f32, name="b3_sb")
    nc.sync.dma_start(out=b3_sb, in_=b3)
    ones_sb = consts.tile([C, 1], f32, name="ones_sb")
    nc.vector.memset(ones_sb, 1.0)

    h1_sb = acts.tile([128, KH, NL], dt_in, name="h1_sb")
    h2_sb = acts.tile([128, KH, NL], dt_in, name="h2_sb")

    def mm_layer(ps, w_t, rhs_sb, kt):
        """Accumulate ps[:, ns] += w_t[:, k].T @ rhs_sb[:, k, ns] over k."""
        for k in range(0, kt, kstep):
            for ns in range(NS):
                if fp8:
                    nc.tensor.matmul(
                        ps[:, ns * 512:(ns + 1) * 512],
                        lhsT=w_t[:, k:k + 2, :],
                        rhs=rhs_sb[:, k:k + 2, ns * 512:(ns + 1) * 512],
                        start=(k == 0),
                        stop=(k + 2 >= kt),
                        perf_mode=perf_mode,
                    )
                else:
                    nc.tensor.matmul(
                        ps[:, ns * 512:(ns + 1) * 512],
                        lhsT=w_t[:, k, :],
                        rhs=rhs_sb[:, k, ns * 512:(ns + 1) * 512],
                        start=(k == 0),
                        stop=(k + 1 >= kt),
                    )

    def finish_early():
        res = epil.tile([1, 1], f32, name="res", tag="res")
        nc.vector.reduce_sum(out=res, in_=h1_sb[0:1, 0, 0:128],
                             axis=mybir.AxisListType.X)
        nc.sync.dma_start(out=out, in_=res)

    if hw_loop and repeat > 1:
        reps = [0]
        loop_cm = tc.For_i(0, repeat, 1,
                           hint_engines=(mybir.EngineType.PE,))
    else:
        reps = range(repeat)
        loop_cm = contextlib.nullcontext()

    with loop_cm:
     for _rep in reps:
        # ---- Layer 1: h1 = relu(X @ W1.T + b1), stored as [j1, i] tiles
        for m in range(KH):
            w1_t = w1p.tile([128, KD, 128], dt_in, name="w1_t", tag="w1t")
            nc.sync.dma_start(out=w1_t, in_=w1[m])
            ps = psum.tile([128, NL], f32, name="ps1", tag="mm")
            mm_layer(ps, w1_t, xt_sb, KD)
            nc.scalar.activation(
                out=h1_sb[:, m, :], in_=ps,
                func=AF.Relu, bias=b1_sb[:, m:m + 1], scale=s12,
            )
        if stage == "l1":
            finish_early()
            continue

        # ---- Layer 2: h2 = relu(h1 @ W2.T + b2)
        for m in range(KH):
            w2_t = w2p.tile([128, KH, 128], dt_in, name="w2_t", tag="w2t")
            nc.sync.dma_start(out=w2_t, in_=w2[m])
            ps = psum.tile([128, NL], f32, name="ps2", tag="mm")
            mm_layer(ps, w2_t, h1_sb, KH)
            nc.scalar.activation(
                out=h2_sb[:, m, :], in_=ps,
                func=AF.Relu, bias=b2_sb[:, m:m + 1], scale=s12,
            )
        if stage == "l2":
            finish_early()
            continue

        # ---- Layer 3: logitsT[c, i] (pre-bias, scaled) in PSUM rows 0..15.
        # Rows 10..15 are zero-weight pad (classes padded to 16 so the
        # dual-fp8 DoubleRow pair-dim step is 16).
        ps3 = psum.tile([128, NL], f32, name="ps3", tag="mm")
        for k in range(0, KH, kstep):
            for ns in range(NS):
                if fp8:
                    nc.tensor.matmul(
                        ps3[0:CP, ns * 512:(ns + 1) * 512],
                        lhsT=w3_sb[:, k:k + 2, :],
                        rhs=h2_sb[:, k:k + 2, ns * 512:(ns + 1) * 512],
                        start=(k == 0),
                        stop=(k + 2 >= KH),
                        perf_mode=perf_mode,
                    )
                else:
                    nc.tensor.matmul(
                        ps3[0:CP, ns * 512:(ns + 1) * 512],
                        lhsT=w3_sb[:, k, :],
                        rhs=h2_sb[:, k, ns * 512:(ns + 1) * 512],
                        start=(k == 0),
                        stop=(k + 1 >= KH),
                    )

        # lg = logitsT + b3 (scalar engine evacuates + rescales PSUM)
        lg = epil.tile([C, NL], f32, name="lg", tag="expT")
        nc.scalar.activation(out=lg, in_=ps3[0:C, :], func=AF.Identity,
                             bias=b3_sb, scale=s3)
        # expT = exp(lg)
        expT = epil.tile([C, NL], f32, name="expT", tag="expT")
        nc.scalar.activation(out=expT, in_=lg, func=AF.Exp)
        if stage == "l3":
            res = epil.tile([1, 1], f32, name="res", tag="res")
            nc.vector.reduce_sum(out=res, in_=expT[0:1, 0:128],
                                 axis=mybir.AxisListType.X)
            nc.sync.dma_start(out=out, in_=res)
            continue

        # pick_b[c] = sum_i lg[c, i] * onehot[c, i] (in-place on lg; lg is
        # not needed afterwards)
        pick_b = epil.tile([C, 1], f32, name="pick_b", tag="pick")
        nc.vector.tensor_tensor(out=lg, in0=lg, in1=oh_sb,
                                op=mybir.AluOpType.mult)
        nc.vector.reduce_sum(out=pick_b, in_=lg, axis=mybir.AxisListType.X)

        # sumexp[1, i] via ones-matmuls over the class partitions
        pse = psum.tile([128, NL], f32, name="pse", tag="mm")
        for ns in range(NS):
            nc.tensor.matmul(
                pse[0:1, ns * 512:(ns + 1) * 512],
                lhsT=ones_sb,
                rhs=expT[:, ns * 512:(ns + 1) * 512],
                start=True, stop=True,
            )
        # lse_tot = sum_i log(sumexp_i), via the Ln activation's accumulator
        lse_tot = epil.tile([1, 1], f32, name="lse_tot", tag="lt")
        nc.scalar.activation(out=lg[0:1, :], in_=pse[0:1, :], func=AF.Ln,
                             accum_out=lse_tot)

        # totals: result = sum_c pick_b[c] - lse_tot
        pt_ps = psum.tile([128, 8], f32, name="pt_ps", tag="mm")
        nc.tensor.matmul(pt_ps[0:1, 0:1], lhsT=ones_sb, rhs=pick_b,
                         start=True, stop=True)
        res = epil.tile([1, 1], f32, name="res", tag="res")
        nc.vector.tensor_tensor(out=res, in0=pt_ps[0:1, 0:1], in1=lse_tot,
                                op=mybir.AluOpType.subtract)
        nc.sync.dma_start(out=out, in_=res)


def _build(repeat=1, stage="full", hw_loop=False, prec="fp8"):
    from contextlib import ExitStack

    import concourse.bacc as bacc
    import concourse.tile as tile
    from concourse import mybir

    f32 = mybir.dt.float32
    dt_in = mybir.dt.float8e4 if prec == "fp8" else mybir.dt.bfloat16

    nc = bacc.Bacc(
        "TRN2",
        target_bir_lowering=False,
        debug=False,
        enable_asserts=False,
        num_devices=N_CORES,
    )
    KD = D // 128
    KH = H // 128
    aps = {
        "xt": nc.dram_tensor("xt", [128, KD, NL], dt_in, kind="ExternalInput").ap(),
        "w1": nc.dram_tensor("w1", [KH, 128, KD, 128], dt_in, kind="ExternalInput").ap(),
        "w2": nc.dram_tensor("w2", [KH, 128, KH, 128], dt_in, kind="ExternalInput").ap(),
        "w3": nc.dram_tensor("w3", [128, KH, CP], dt_in, kind="ExternalInput").ap(),
        "b1": nc.dram_tensor("b1", [128, KH], f32, kind="ExternalInput").ap(),
        "b2": nc.dram_tensor("b2", [128, KH], f32, kind="ExternalInput").ap(),
        "b3": nc.dram_tensor("b3", [C, 1], f32, kind="ExternalInput").ap(),
        "oh": nc.dram_tensor("oh", [C, NL], f32, kind="ExternalInput").ap(),
        "out": nc.dram_tensor("out", [1, 1], f32, kind="ExternalOutput").ap(),
    }
    with tile.TileContext(nc) as tc:
        with ExitStack() as ctx:
            _emit(tc, ctx, aps, repeat, stage, hw_loop, prec)
    nc.compile()
    return nc


def _q8(x, s):
    return np.clip(x.astype(np.float32) * s, -240.0, 240.0).astype(E4M3)


def prep_inputs(X, Y, W1, b1, W2, b2, W3, b3, prec="fp8"):
    """Shard + retile (+ scale/quantize) the full inputs into per-core maps."""
    KD = D // 128
    KH = H // 128
    fp8 = prec == "fp8"

    if fp8:
        W1c = _q8(W1, SW)
        W2c = _q8(W2, SW)
        W3c = _q8(W3, SW)
        b1c = (b1.astype(np.float32) * SH)
        b2c = (b2.astype(np.float32) * SH)
    else:
        W1c, W2c, W3c = W1.astype(BF16), W2.astype(BF16), W3.astype(BF16)
        b1c, b2c = b1.astype(np.float32), b2.astype(np.float32)

    w1p = np.ascontiguousarray(W1c.reshape(KH, 128, KD, 128).transpose(0, 3, 2, 1))
    w2p = np.ascontiguousarray(W2c.reshape(KH, 128, KH, 128).transpose(0, 3, 2, 1))
    W3pad = np.zeros((CP, H), dtype=W3c.dtype)
    W3pad[:C] = W3c
    w3p = np.ascontiguousarray(W3pad.reshape(CP, KH, 128).transpose(2, 1, 0))
    b1p = np.ascontiguousarray(b1c.reshape(KH, 128).T)
    b2p = np.ascontiguousarray(b2c.reshape(KH, 128).T)
    b3p = np.ascontiguousarray(b3.astype(np.float32).reshape(C, 1))

    Xb = _q8(X, SX) if fp8 else X.astype(BF16)
    in_maps = []
    for c in range(N_CORES):
        Xc = Xb[c * NL:(c + 1) * NL]
        xtp = np.ascontiguousarray(Xc.reshape(NL, KD, 128).transpose(2, 1, 0))
        Yc = Y[c * NL:(c + 1) * NL]
        ohp = (np.arange(C, dtype=np.int64)[:, None] == Yc[None, :].astype(np.int64))
        ohp = np.ascontiguousarray(ohp.astype(np.float32))
        in_maps.append({
            "xt": xtp, "w1": w1p, "w2": w2p, "w3": w3p,
            "b1": b1p, "b2": b2p, "b3": b3p, "oh": ohp,
        })
    return in_maps


def log_prior(W1, b1, W2, b2, W3, b3):
    params = (W1, b1, W2, b2, W3, b3)
    d = sum(p.size for p in params)
    sq = sum(float(np.sum(p.astype(np.float64) ** 2)) for p in params)
    return -0.5 * (sq / PRIOR_VAR + d * math.log(2.0 * math.pi * PRIOR_VAR))


def _get_nc(repeat=1, hw_loop=False, prec="fp8"):
    key = (repeat, hw_loop, prec)
    if key not in _compiled:
        _compiled[key] = _build(repeat, hw_loop=hw_loop, prec=prec)
    return _compiled[key]


def run_device(in_maps, repeat=1, prec="fp8"):
    from concourse.bass_utils import run_bass_kernel_spmd

    nc = _get_nc(repeat, prec=prec)
    res = run_bass_kernel_spmd(nc, in_maps, list(range(N_CORES)))
    return [r["out"][0, 0] for r in res.results]


def kernel(X, Y, W1, b1, W2, b2, W3, b3):
    X = np.asarray(X)
    Y = np.asarray(Y)
    W1 = np.asarray(W1)
    b1 = np.asarray(b1)
    W2 = np.asarray(W2)
    b2 = np.asarray(b2)
    W3 = np.asarray(W3)
    b3 = np.asarray(b3)

    try:
        in_maps = prep_inputs(X, Y, W1, b1, W2, b2, W3, b3, prec="fp8")
        partials = run_device(in_maps, prec="fp8")
    except Exception:
        # Safety net: fp8 DoubleRow leans on newer walrus/ISA behavior; the
        # bf16 path is plain matmuls.
        in_maps = prep_inputs(X, Y, W1, b1, W2, b2, W3, b3, prec="bf16")
        partials = run_device(in_maps, prec="bf16")
    total = float(np.sum(np.asarray(partials, dtype=np.float64)))
    total += log_prior(W1, b1, W2, b2, W3, b3)
    return np.array(total, dtype=np.float32)


# revision 50
# speedup vs baseline: 53.1153x; 1.1852x over previous
"""Trainium2 Bass kernel for the dense-MLP Bayesian log-joint problem.

Computes, for fixed MLP weights:
    h1 = relu(X @ W1.T + b1); h2 = relu(h1 @ W2.T + b2)
    logits = h2 @ W3.T + b3
    out = sum_i log_softmax(logits)[i, Y[i]] + log MVN(0, 100 I)(params)

Strategy: data-parallel over 8 NeuronCores. Each core gets 2048 rows of
X/Y plus a replicated copy of the (small) weights, computes its partial
log-likelihood sum on-device, and the host adds the partials plus the
closed-form Gaussian prior term.

On-device layout is "transposed activations": every matmul keeps the
contraction dim on SBUF partitions. The host pre-transposes X and the
weight matrices into PE-friendly tiles so no on-device transposes are
needed.

Matmuls run in fp8 (e4m3) with DoubleRow perf mode: inputs are scaled by
powers of two into fp8 range on the host, and the PSUM results are
rescaled exactly inside the (fp32) activation that applies bias+relu.
fp32 PSUM accumulation throughout; the log-softmax epilogue is fp32.
The final scalar is dominated by the prior constant d*log(2*pi*100), so
the quantized forward error lands at ~1e-7 relative (measured 5.7e-8
against an f64 reference on the real inputs; vs the f32 jax reference
both fp8 and bf16 modes measure 0.0 relative error).

Measured on 8 axon TRN2 cores (hardware For_i loop, paired trip-count
differencing): ~211 us per full evaluation, ~983 TFLOP/s aggregate
(~78% of theoretical fp8 peak; bf16 mode: ~473 us).
"""

import math

import numpy as np
import ml_dtypes

N = 16384
D = 1024
H = 2048
C = 10
CP = 16  # classes padded to 16 so layer-3 DoubleRow satisfies step%16==0
N_CORES = 8
NL = N // N_CORES  # 2048 rows per core
PRIOR_VAR = 100.0

BF16 = ml_dtypes.bfloat16
E4M3 = ml_dtypes.float8_e4m3  # TRN fp8e4: max normal +-240

# Power-of-two scales that place X / weights / hidden activations into
# fp8e4m3's sweet spot. All rescales are exact in fp32.
SX = 16.0
SW = 128.0
SH = 16.0

_compiled = {}


def _emit(tc, ctx, aps, repeat, stage="full", hw_loop=False, prec="fp8"):
    import contextlib

    import concourse.bass as bass
    from concourse import mybir

    nc = tc.nc
    f32 = mybir.dt.float32
    AF = mybir.ActivationFunctionType
    fp8 = prec == "fp8"
    dt_in = mybir.dt.float8e4 if fp8 else mybir.dt.bfloat16
    perf_mode = mybir.MatmulPerfMode.DoubleRow if fp8 else None
    kstep = 2 if fp8 else 1
    # PSUM -> activation rescales (exact powers of two)
    s12 = SH / (SX * SW) if fp8 else 1.0   # layer1 out scale; layer2 identical
    s2 = SH / (SH * SW) if fp8 else 1.0
    s3 = 1.0 / (SH * SW) if fp8 else 1.0

    xt, w1, w2, w3, b1, b2, b3, oh, out = (
        aps["xt"], aps["w1"], aps["w2"], aps["w3"],
        aps["b1"], aps["b2"], aps["b3"], aps["oh"], aps["out"],
    )

    KD = D // 128   # 8  k-tiles for layer 1
    KH = H // 128   # 16 k-tiles for layers 2/3, and m-tiles for layers 1/2
    NS = NL // 512  # 4  n-slices of the batch free dim

    consts = ctx.enter_context(tc.tile_pool(name="consts", bufs=1))
    acts = ctx.enter_context(tc.tile_pool(name="acts", bufs=1))
    w1p = ctx.enter_context(tc.tile_pool(name="w1p", bufs=3))
    w2p = ctx.enter_context(tc.tile_pool(name="w2p", bufs=3))
    psum = ctx.enter_context(tc.tile_pool(name="psum", bufs=2, space="PSUM"))
    epil = ctx.enter_context(tc.tile_pool(name="epil", bufs=2))

    # Constants / resident tensors
    xt_sb = consts.tile([128, KD, NL], dt_in, name="xt_sb")
    for kd in range(KD):
        nc.sync.dma_start(out=xt_sb[:, kd, :], in_=xt[:, kd, :])
    w3_sb = consts.tile([128, KH, CP], dt_in, name="w3_sb")
    nc.sync.dma_start(out=w3_sb, in_=w3)
    oh_sb = consts.tile([C, NL], f32, name="oh_sb")
    nc.sync.dma_start(out=oh_sb, in_=oh)
    b1_sb = consts.tile([128, KH], f32, name="b1_sb")
    nc.sync.dma_start(out=b1_sb, in_=b1)
    b2_sb = consts.tile([128, KH], f32, name="b2_sb")
    nc.sync.dma_start(out=b2_sb, in_=b2)
    b3_sb = consts.tile([C, 1], f32, name="b3_sb")
    nc.sync.dma_start(out=b3_sb, in_=b3)
    ones_sb = consts.tile([C, 1], f32, name="ones_sb")
    nc.vector.memset(ones_sb, 1.0)

    h1_sb = acts.tile([128, KH, NL], dt_in, name="h1_sb")
    h2_sb = acts.tile([128, KH, NL], dt_in, name="h2_sb")

    def mm_layer(ps, w_t, rhs_sb, kt):
        """Accumulate ps[:, ns] += w_t[:, k].T @ rhs_sb[:, k, ns] over k."""
        for k in range(0, kt, kstep):
            for ns in range(NS):
                if fp8:
                    nc.tensor.matmul(
                        ps[:, ns * 512:(ns + 1) * 512],
                        lhsT=w_t[:, k:k + 2, :],
                        rhs=rhs_sb[:, k:k + 2, ns * 512:(ns + 1) * 512],
                        start=(k == 0),
                        stop=(k + 2 >= kt),
                        perf_mode=perf_mode,
                    )
                else:
                    nc.tensor.matmul(
                        ps[:, ns * 512:(ns + 1) * 512],
                        lhsT=w_t[:, k, :],
                        rhs=rhs_sb[:, k, ns * 512:(ns + 1) * 512],
                        start=(k == 0),
                        stop=(k + 1 >= kt),
                    )

    def finish_early():
        res = epil.tile([1, 1], f32, name="res", tag="res")
        nc.vector.reduce_sum(out=res, in_=h1_sb[0:1, 0, 0:128],
                             axis=mybir.AxisListType.X)
        nc.sync.dma_start(out=out, in_=res)

    if hw_loop and repeat > 1:
        reps = [0]
        loop_cm = tc.For_i(0, repeat, 1,
                           hint_engines=(mybir.EngineType.PE,))
    else:
        reps = range(repeat)
        loop_cm = contextlib.nullcontext()

    with loop_cm:
     for _rep in reps:
        # ---- Layer 1: h1 = relu(X @ W1.T + b1), stored as [j1, i] tiles
        for m in range(KH):
            w1_t = w1p.tile([128, KD, 128], dt_in, name="w1_t", tag="w1t")
            nc.sync.dma_start(out=w1_t, in_=w1[m])
            ps = psum.tile([128, NL], f32, name="ps1", tag="mm")
            mm_layer(ps, w1_t, xt_sb, KD)
            nc.scalar.activation(
                out=h1_sb[:, m, :], in_=ps,
                func=AF.Relu, bias=b1_sb[:, m:m + 1], scale=s12,
            )
        if stage == "l1":
            finish_early()
            continue

        # ---- Layer 2: h2 = relu(h1 @ W2.T + b2)
        for m in range(KH):
            w2_t = w2p.tile([128, KH, 128], dt_in, name="w2_t", tag="w2t")
            nc.sync.dma_start(out=w2_t, in_=w2[m])
            ps = psum.tile([128, NL], f32, name="ps2", tag="mm")
            mm_layer(ps, w2_t, h1_sb, KH)
            nc.scalar.activation(
                out=h2_sb[:, m, :], in_=ps,
                func=AF.Relu, bias=b2_sb[:, m:m + 1], scale=s12,
            )
        if stage == "l2":
            finish_early()
            continue

        # ---- Layer 3: logitsT[c, i] (pre-bias, scaled) in PSUM rows 0..15.
        # Rows 10..15 are zero-weight pad (classes padded to 16 so the
        # dual-fp8 DoubleRow pair-dim step is 16).
        ps3 = psum.tile([128, NL], f32, name="ps3", tag="mm")
        for k in range(0, KH, kstep):
            for ns in range(NS):
                if fp8:
                    nc.tensor.matmul(
                        ps3[0:CP, ns * 512:(ns + 1) * 512],
                        lhsT=w3_sb[:, k:k + 2, :],
                        rhs=h2_sb[:, k:k + 2, ns * 512:(ns + 1) * 512],
                        start=(k == 0),
                        stop=(k + 2 >= KH),
                        perf_mode=perf_mode,
                    )
                else:
                    nc.tensor.matmul(
                        ps3[0:CP, ns * 512:(ns + 1) * 512],
                        lhsT=w3_sb[:, k, :],
                        rhs=h2_sb[:, k, ns * 512:(ns + 1) * 512],
                        start=(k == 0),
                        stop=(k + 1 >= KH),
                    )

        # lg = logitsT + b3 (scalar engine evacuates + rescales PSUM)
        lg = epil.tile([C, NL], f32, name="lg", tag="expT")
        nc.scalar.activation(out=lg, in_=ps3[0:C, :], func=AF.Identity,
                             bias=b3_sb, scale=s3)
        # expT = exp(lg)
        expT = epil.tile([C, NL], f32, name="expT", tag="expT")
        nc.scalar.activation(out=expT, in_=lg, func=AF.Exp)

        # pick_b[c] = sum_i lg[c, i] * onehot[c, i] (in-place on lg; lg is
        # not needed afterwards)
        pick_b = epil.tile([C, 1], f32, name="pick_b", tag="pick")
        nc.vector.tensor_tensor(out=lg, in0=lg, in1=oh_sb,
                                op=mybir.AluOpType.mult)
        nc.vector.reduce_sum(out=pick_b, in_=lg, axis=mybir.AxisListType.X)

        # sumexp[1, i] via ones-matmuls over the class partitions
        pse = psum.tile([128, NL], f32, name="pse", tag="mm")
        for ns in range(NS):
            nc.tensor.matmul(
                pse[0:1, ns * 512:(ns + 1) * 512],
                lhsT=ones_sb,
                rhs=expT[:, ns * 512:(ns + 1) * 512],
                start=True, stop=True,
            )
        # lse_tot = sum_i log(sumexp_i), via the Ln activation's accumulator
        lse_tot = epil.tile([1, 1], f32, name="lse_tot", tag="lt")
        nc.scalar.activation(out=lg[0:1, :], in_=pse[0:1, :], func=AF.Ln,
                             accum_out=lse_tot)

        # totals: result = sum_c pick_b[c] - lse_tot
        pt_ps = psum.tile([128, 8], f32, name="pt_ps", tag="mm")
        nc.tensor.matmul(pt_ps[0:1, 0:1], lhsT=ones_sb, rhs=pick_b,
                         start=True, stop=True)
        res = epil.tile([1, 1], f32, name="res", tag="res")
        nc.vector.tensor_tensor(out=res, in0=pt_ps[0:1, 0:1], in1=lse_tot,
                                op=mybir.AluOpType.subtract)
        nc.sync.dma_start(out=out, in_=res)


def _build(repeat=1, stage="full", hw_loop=False, prec="fp8"):
    from contextlib import ExitStack

    import concourse.bacc as bacc
    import concourse.tile as tile
    from concourse import mybir

    f32 = mybir.dt.float32
    dt_in = mybir.dt.float8e4 if prec == "fp8" else mybir.dt.bfloat16

    nc = bacc.Bacc(
        "TRN2",
        target_bir_lowering=False,
        debug=False,
        enable_asserts=False,
        num_devices=N_CORES,
    )
    KD = D // 128
    KH = H // 128
    aps = {
        "xt": nc.dram_tensor("xt", [128, KD, NL], dt_in, kind="ExternalInput").ap(),
        "w1": nc.dram_tensor("w1", [KH, 128, KD, 128], dt_in, kind="ExternalInput").ap(),
        "w2": nc.dram_tensor("w2", [KH, 128, KH, 128], dt_in, kind="ExternalInput").ap(),
        "w3": nc.dram_tensor("w3", [128, KH, CP], dt_in, kind="ExternalInput").ap(),
        "b1": nc.dram_tensor("b1", [128, KH], f32, kind="ExternalInput").ap(),
        "b2": nc.dram_tensor("b2", [128, KH], f32, kind="ExternalInput").ap(),
        "b3": nc.dram_tensor("b3", [C, 1], f32, kind="ExternalInput").ap(),
        "oh": nc.dram_tensor("oh", [C, NL], f32, kind="ExternalInput").ap(),
        "out": nc.dram_tensor("out", [1, 1], f32, kind="ExternalOutput").ap(),
    }
    with tile.TileContext(nc) as tc:
        with ExitStack() as ctx:
            _emit(tc, ctx, aps, repeat, stage, hw_loop, prec)
    nc.compile()
    return nc


def _q8(x, s):
    return np.clip(x.astype(np.float32) * s, -240.0, 240.0).astype(E4M3)


def prep_inputs(X, Y, W1, b1, W2, b2, W3, b3, prec="fp8"):
    """Shard + retile (+ scale/quantize) the full inputs into per-core maps."""
    KD = D // 128
    KH = H // 128
    fp8 = prec == "fp8"

    if fp8:
        W1c = _q8(W1, SW)
        W2c = _q8(W2, SW)
        W3c = _q8(W3, SW)
        b1c = (b1.astype(np.float32) * SH)
        b2c = (b2.astype(np.float32) * SH)
    else:
        W1c, W2c, W3c = W1.astype(BF16), W2.astype(BF16), W3.astype(BF16)
        b1c, b2c = b1.astype(np.float32), b2.astype(np.float32)

    w1p = np.ascontiguousarray(W1c.reshape(KH, 128, KD, 128).transpose(0, 3, 2, 1))
    w2p = np.ascontiguousarray(W2c.reshape(KH, 128, KH, 128).transpose(0, 3, 2, 1))
    W3pad = np.zeros((CP, H), dtype=W3c.dtype)
    W3pad[:C] = W3c
    w3p = np.ascontiguousarray(W3pad.reshape(CP, KH, 128).transpose(2, 1, 0))
    b1p = np.ascontiguousarray(b1c.reshape(KH, 128).T)
    b2p = np.ascontiguousarray(b2c.reshape(KH, 128).T)
    b3p = np.ascontiguousarray(b3.astype(np.float32).reshape(C, 1))

    Xb = _q8(X, SX) if fp8 else X.astype(BF16)
    in_maps = []
    for c in range(N_CORES):
        Xc = Xb[c * NL:(c + 1) * NL]
        xtp = np.ascontiguousarray(Xc.reshape(NL, KD, 128).transpose(2, 1, 0))
        Yc = Y[c * NL:(c + 1) * NL]
        ohp = (np.arange(C, dtype=np.int64)[:, None] == Yc[None, :].astype(np.int64))
        ohp = np.ascontiguousarray(ohp.astype(np.float32))
        in_maps.append({
            "xt": xtp, "w1": w1p, "w2": w2p, "w3": w3p,
            "b1": b1p, "b2": b2p, "b3": b3p, "oh": ohp,
        })
    return in_maps


def log_prior(W1, b1, W2, b2, W3, b3):
    params = (W1, b1, W2, b2, W3, b3)
    d = sum(p.size for p in params)
    sq = sum(float(np.sum(p.astype(np.float64) ** 2)) for p in params)
    return -0.5 * (sq / PRIOR_VAR + d * math.log(2.0 * math.pi * PRIOR_VAR))


def _get_nc(repeat=1, hw_loop=False, prec="fp8"):
    key = (repeat, hw_loop, prec)
    if key not in _compiled:
        _compiled[key] = _build(repeat, hw_loop=hw_loop, prec=prec)
    return _compiled[key]


def run_device(in_maps, repeat=1, prec="fp8"):
    from concourse.bass_utils import run_bass_kernel_spmd

    nc = _get_nc(repeat, prec=prec)
    res = run_bass_kernel_spmd(nc, in_maps, list(range(N_CORES)))
    return [r["out"][0, 0] for r in res.results]


def kernel(X, Y, W1, b1, W2, b2, W3, b3):
    X = np.asarray(X)
    Y = np.asarray(Y)
    W1 = np.asarray(W1)
    b1 = np.asarray(b1)
    W2 = np.asarray(W2)
    b2 = np.asarray(b2)
    W3 = np.asarray(W3)
    b3 = np.asarray(b3)

    try:
        in_maps = prep_inputs(X, Y, W1, b1, W2, b2, W3, b3, prec="fp8")
        partials = run_device(in_maps, prec="fp8")
    except Exception:
        # Safety net: fp8 DoubleRow leans on newer walrus/ISA behavior; the
        # bf16 path is plain matmuls.
        in_maps = prep_inputs(X, Y, W1, b1, W2, b2, W3, b3, prec="bf16")
        partials = run_device(in_maps, prec="bf16")
    total = float(np.sum(np.asarray(partials, dtype=np.float64)))
    total += log_prior(W1, b1, W2, b2, W3, b3)
    return np.array(total, dtype=np.float32)
